# revision 106
# baseline (speedup 1.0000x reference)
"""GAT 2-layer kernel for 8 Trainium2 NeuronCores (bf16 pipeline).

Strategy (edge-parallel over dst-sorted edges, node-range sharded): host
appends self-loops, sorts edges by dst, gives each core a contiguous 6250-dst
range split into windows; each window's edges fill 10 tiles of 128 slots
(5 "lo" + 5 "hi" tiles split by src index so int16 dma_gather indices reach
the whole node table). Per-slot attention logits alpha = a_src[src] +
a_dst[dst] are host-expanded (bf16), like all index prep.

  - Launch T: [xh | a_src | a_dst] = x^T-tiles @ [W1P | W1A] per core from a
    host-pretransposed bf16 xT; psums grouped 3 tiles per bank, psum->SBUF
    copies alternate ACT/DVE, one DMA in / six piece DMAs out.
  - Launch E1 (heads=8, 125-dst windows, 5-window chunks): per chunk,
    dma_gather of bf16 xh rows (256B); e = exp(leaky(alpha)) on ACT;
    msg = xh[src] * e on DVE (2x, c-major head broadcast); one-hot S per tile
    (tensor_scalar is_equal, 4x); segment sums via S^T @ [msg | e] matmuls
    accumulated in PSUM; ACT copies psums to a bf16 chunk buffer; the
    normalize + bias + ELU epilogue runs batched over the chunk one chunk
    behind (software pipelining, so the in-order DVE queue never stalls);
    batched PE transposes + [h@W2 | h@W2A] matmuls; per-chunk output DMAs.
    The final chunk's epilogue runs in two overlapping halves to shorten the
    drain. Host reassembles the layer-2 table between launches.
  - Launch E2 (heads=1, 128-dst windows, 7-window chunks -> fewer gather
    calls since E2 is Pool/desc-gen bound): e2 is folded into the selection
    matrix (S_e = e2 * one_hot via fused is_equal+mult), the gathered 512B
    rows carry a trailing 1.0 so one matmul yields [agg | s]; divide-by-s is
    fused into the ACT psum copy as a per-partition scale; + b2; per-chunk
    stores (per-window in the ragged last chunk).

Sharding note (vs the edge-parallel hint): edges are sharded by dst range so
all segment reductions stay core-local in PSUM - no cross-core all-reduce is
needed; the small weights are folded/replicated on the host side.
"""

import os
import sys

sys.path.insert(0, "/opt/trn_rl_repo")

import numpy as np
import ml_dtypes

import concourse.bass as bass
import concourse.bacc as bacc
import concourse.mybir as mybir
import concourse.tile as tile
from concourse.bass_utils import run_bass_kernel_spmd

F32 = mybir.dt.float32
BF16 = mybir.dt.bfloat16
I16 = mybir.dt.int16
ALU = mybir.AluOpType
ACTF = mybir.ActivationFunctionType
BF = ml_dtypes.bfloat16

# Problem constants (hardcoded per harness contract).
N = 50000
E = 400000
FIN = 128
H1, C1 = 8, 16          # layer-1 heads / channels
FMID = H1 * C1          # 128
FOUT = 128
NEG_SLOPE = 0.2

NCORES = 8
NPC = N // NCORES       # 6250 nodes per core
LOT = 5                 # lo tiles per window (src < 32768 reachable)
HIT = 5                 # hi tiles per window (src >= HI_BASE reachable)
TPW = LOT + HIT         # 10 tiles of 128 slots per window
SENT = -1               # sentinel dst_rel for padding slots
HI_BASE = N - 32768     # 17232: hi gather covers rows [HI_BASE, N)
NT_T = (NPC + 127) // 128  # x tiles per core in launch T (49)
NPC_PAD = NT_T * 128
TCOLS = FMID + 2 * H1   # 144: [xh | a_src | a_dst] in launch T


class Layout:
    """Per-launch window/chunk geometry. E1 is DVE-bound and prefers small
    chunks (tight pipelining); E2 is Pool-bound and prefers fewer, larger
    windows (fewer gather calls)."""

    def __init__(self, win, wins, chunk_w):
        self.WIN = win
        self.WINS = wins
        self.CW = chunk_w
        self.CHUNKS = wins // chunk_w
        self.TPC = chunk_w * TPW
        self.LO_N = chunk_w * LOT * 128
        self.HI_N = chunk_w * HIT * 128
        self.NTILES = self.CHUNKS * self.TPC


LAY1 = Layout(125, 50, 5)
LAY2 = Layout(128, 49, 7)

_CACHE = {}


# ----------------------------------------------------------------------------
# Host-side graph preprocessing
# ----------------------------------------------------------------------------

def _wrap16(idx):
    """int16 index array [n] -> dma_gather wrapped layout [16, n//16]."""
    n = idx.shape[0]
    return np.ascontiguousarray(idx.reshape(n // 16, 16).T.astype(np.int16))


def _prep_edges(src, dst, lay):
    """Returns per-core dicts with device index arrays and host slot maps.

    Chunk slot layout: tile g of chunk ch is lo-block [w'*LOT + t] for
    w'=g//LOT when g < CW*LOT else hi-block. Slot i of a gather call
    lands at [i % 128, i // 128] of the call's tile range.
    """
    s_all = np.concatenate([src, np.arange(N, dtype=np.int64)])
    d_all = np.concatenate([dst, np.arange(N, dtype=np.int64)])
    order = np.argsort(d_all, kind="stable")
    s_all = s_all[order]
    d_all = d_all[order]
    counts = np.bincount(d_all, minlength=N)
    starts = np.concatenate([[0], np.cumsum(counts)])
    cores = []
    for c in range(NCORES):
        ilo = np.zeros((lay.CHUNKS, lay.LO_N), np.int64)
        ihi = np.zeros((lay.CHUNKS, lay.HI_N), np.int64)
        slot_src = np.zeros((lay.NTILES, 128), np.int64)
        slot_dst = np.zeros((lay.NTILES, 128), np.int64)
        slot_rel = np.full((lay.NTILES, 128), SENT, np.int64)
        for ch in range(lay.CHUNKS):
            lo_flat = np.zeros(lay.LO_N, np.int64)
            hi_flat = np.full(lay.HI_N, HI_BASE, np.int64)
            for wi in range(lay.CW):
                w = ch * lay.CW + wi
                base = c * NPC + w * lay.WIN
                wend = min(base + lay.WIN, (c + 1) * NPC)
                e0, e1 = starts[base], starts[wend]
                ss, dd = s_all[e0:e1], d_all[e0:e1]
                must_lo = ss < HI_BASE
                must_hi = ss >= 32768
                free = ~must_lo & ~must_hi
                n_lo = int(must_lo.sum())
                cap_lo = LOT * 128
                take = min(int(free.sum()), cap_lo - n_lo)
                sel_lo = must_lo.copy()
                free_idx = np.where(free)[0]
                sel_lo[free_idx[:take]] = True
                sel_hi = ~sel_lo
                nl, nh = int(sel_lo.sum()), int(sel_hi.sum())
                assert nl <= cap_lo and nh <= HIT * 128, (nl, nh)
                # lo block
                ls = np.zeros(cap_lo, np.int64)
                ld = np.zeros(cap_lo, np.int64)
                lr = np.full(cap_lo, SENT, np.int64)
                ls[:nl] = ss[sel_lo]
                ld[:nl] = dd[sel_lo]
                lr[:nl] = dd[sel_lo] - base
                lo_flat[wi * cap_lo:(wi + 1) * cap_lo] = ls
                g0 = ch * lay.TPC + wi * LOT
                slot_src[g0:g0 + LOT] = ls.reshape(LOT, 128)
                slot_dst[g0:g0 + LOT] = ld.reshape(LOT, 128)
                slot_rel[g0:g0 + LOT] = lr.reshape(LOT, 128)
                # hi block
                cap_hi = HIT * 128
                hs = np.full(cap_hi, HI_BASE, np.int64)
                hd = np.zeros(cap_hi, np.int64)
                hr = np.full(cap_hi, SENT, np.int64)
                hs[:nh] = ss[sel_hi]
                hd[:nh] = dd[sel_hi]
                hr[:nh] = dd[sel_hi] - base
                hi_flat[wi * cap_hi:(wi + 1) * cap_hi] = hs
                g1 = ch * lay.TPC + lay.CW * LOT + wi * HIT
                slot_src[g1:g1 + HIT] = hs.reshape(HIT, 128)
                slot_dst[g1:g1 + HIT] = hd.reshape(HIT, 128)
                slot_rel[g1:g1 + HIT] = hr.reshape(HIT, 128)
            ilo[ch] = lo_flat
            ihi[ch] = hi_flat - HI_BASE
        idx_lo = np.concatenate([_wrap16(ilo[ch]) for ch in range(lay.CHUNKS)],
                                axis=1)
        idx_hi = np.concatenate([_wrap16(ihi[ch]) for ch in range(lay.CHUNKS)],
                                axis=1)
        pad_lo = np.ascontiguousarray(np.tile(idx_lo, (8, 1)))
        pad_hi = np.ascontiguousarray(np.tile(idx_hi, (8, 1)))
        cores.append({
            "idx_lo": pad_lo, "idx_hi": pad_hi,
            "slot_src": np.ascontiguousarray(slot_src.T),   # [128, n_tiles]
            "slot_dst": np.ascontiguousarray(slot_dst.T),
            "drel": np.ascontiguousarray(slot_rel.T.astype(np.float32)),
        })
    return cores


def _perm_cmajor():
    """Column permutation h*16+c -> c*8+h for layer-1 features."""
    p = np.zeros(FMID, np.int64)
    for h in range(H1):
        for c in range(C1):
            p[c * H1 + h] = h * C1 + c
    return p


# ----------------------------------------------------------------------------
# Bass program builders
# ----------------------------------------------------------------------------

def _new_nc():
    return bacc.Bacc("TRN2", target_bir_lowering=False, debug=False,
                     num_devices=NCORES)


def build_T():
    """Table launch: [xh | a_src | a_dst] = xT^T @ [W1P | W1A] per core."""
    nc = _new_nc()
    xt_in = nc.declare_dram_parameter("xt", [128, NPC_PAD], BF16, isOutput=False)
    w_in = nc.declare_dram_parameter("w", [FIN, TCOLS], BF16, isOutput=False)
    dump_out = nc.declare_dram_parameter("dump", [128, NT_T * TCOLS], BF16,
                                         isOutput=True)

    with tile.TileContext(nc) as tc:
        with (
            tc.tile_pool(name="const", bufs=1) as cpool,
            tc.tile_pool(name="ps", bufs=6, space="PSUM") as pspool,
        ):
            xt = cpool.tile([128, NPC_PAD], BF16)
            w = cpool.tile([FIN, TCOLS], BF16)
            acc = cpool.tile([128, NT_T, TCOLS], BF16)
            # split xT load so tile-0 compute starts early
            nc.sync.dma_start(out=w[:], in_=w_in[:, :])
            q = [0, 6 * 128, 18 * 128, 34 * 128, NPC_PAD]
            for i in range(4):
                nc.sync.dma_start(out=xt[:, q[i]:q[i + 1]],
                                  in_=xt_in[:, q[i]:q[i + 1]])
            for g0 in range(0, NT_T, 3):
                gn = min(3, NT_T - g0)
                ps = pspool.tile([128, 3, TCOLS], F32, space="PSUM")
                for j in range(gn):
                    t = g0 + j
                    nc.tensor.matmul(out=ps[:, j, :],
                                     lhsT=xt[:, t * 128:(t + 1) * 128],
                                     rhs=w[:], start=True, stop=True)
                if (g0 // 3) % 2 == 0:
                    nc.scalar.copy(out=acc[:, g0:g0 + gn, :], in_=ps[:, 0:gn, :])
                else:
                    nc.vector.tensor_copy(out=acc[:, g0:g0 + gn, :],
                                          in_=ps[:, 0:gn, :])
                if g0 + gn in (9, 18, 27, 36, 42, NT_T):
                    marks = [0, 9, 18, 27, 36, 42, NT_T]
                    d0 = marks[marks.index(g0 + gn) - 1] * TCOLS
                    d1 = (g0 + gn) * TCOLS
                    nc.sync.dma_start(out=dump_out[:, d0:d1],
                                      in_=acc[:, d0 // TCOLS:(g0 + gn), :])
    nc.compile()
    return nc


def _emit_gathers(nc, G, table_in, idx, base_tile, n_tiles, idx_col0):
    # dma_gather is limited to 1024 idxs (64 descs/engine packet)
    done = 0
    while done < n_tiles:
        piece = min(8, n_tiles - done)
        nidx = piece * 128
        c0 = idx_col0 + done * 8
        nc.gpsimd.dma_gather(
            out_ap=G[:, base_tile + done:base_tile + done + piece, :],
            in_ap=table_in, idxs_ap=idx[:, c0:c0 + nidx // 16],
            num_idxs=nidx, num_idxs_reg=nidx,
            elem_size=table_in.shape[-1])
        done += piece


def _emit_gathers_il(nc, G, lo_ap, hi_ap, ilo, ihi, nlo_t, nhi_t,
                     lo_c0, hi_c0):
    """Interleave lo/hi gather pieces so each window's full tile set (its lo
    AND hi block) lands as early as possible."""
    lo_done = hi_done = 0
    while lo_done < nlo_t or hi_done < nhi_t:
        for ap, idx, done, n_t, c0, base in (
                (lo_ap, ilo, lo_done, nlo_t, lo_c0, 0),
                (hi_ap, ihi, hi_done, nhi_t, hi_c0, nlo_t)):
            if done >= n_t:
                continue
            piece = min(8, n_t - done)
            nidx = piece * 128
            cc = c0 + done * 8
            nc.gpsimd.dma_gather(
                out_ap=G[:, base + done:base + done + piece, :],
                in_ap=ap, idxs_ap=idx[:, cc:cc + nidx // 16],
                num_idxs=nidx, num_idxs_reg=nidx,
                elem_size=ap.shape[-1])
        lo_done = min(nlo_t, lo_done + 8)
        hi_done = min(nhi_t, hi_done + 8)


def build_E1():
    lay = LAY1
    CHUNKS, TPC, LO_N, HI_N, NTILES = (lay.CHUNKS, lay.TPC, lay.LO_N,
                                       lay.HI_N, lay.NTILES)
    CHUNK_W, WINS, WIN_NODES = lay.CW, lay.WINS, lay.WIN
    nc = _new_nc()
    table_in = nc.declare_dram_parameter("table", [N, 128], BF16, isOutput=False)
    ae_in = nc.declare_dram_parameter("ae", [128, NTILES, H1], BF16,
                                      isOutput=False)
    ilo_in = nc.declare_dram_parameter("ilo", [128, CHUNKS * LO_N // 16], I16,
                                       isOutput=False)
    ihi_in = nc.declare_dram_parameter("ihi", [128, CHUNKS * HI_N // 16], I16,
                                       isOutput=False)
    drel_in = nc.declare_dram_parameter("drel", [128, NTILES], F32, isOutput=False)
    iota_in = nc.declare_dram_parameter("iota", [128, 128], BF16, isOutput=False)
    b1_in = nc.declare_dram_parameter("b1rep", [128, FMID], BF16, isOutput=False)
    id_in = nc.declare_dram_parameter("ident", [128, 128], BF16, isOutput=False)
    w2c_in = nc.declare_dram_parameter("w2c", [FMID, FOUT + 2], BF16,
                                       isOutput=False)
    dump_out = nc.declare_dram_parameter("dump", [128, WINS * (FOUT + 2)], BF16,
                                         isOutput=True)

    with tile.TileContext(nc) as tc:
        with (
            tc.tile_pool(name="const", bufs=1) as cpool,
            tc.tile_pool(name="gat", bufs=2) as gpool,
            tc.tile_pool(name="alp", bufs=2) as apool,
            tc.tile_pool(name="rhs", bufs=2) as rpool,
            tc.tile_pool(name="sel", bufs=24) as spool,
            tc.tile_pool(name="psw", bufs=2, space="PSUM") as ppool,
            tc.tile_pool(name="accp", bufs=2) as accppool,
            tc.tile_pool(name="acca", bufs=2) as accapool,
            tc.tile_pool(name="epi", bufs=2) as epool,
            tc.tile_pool(name="hel", bufs=2) as hpool,
            tc.tile_pool(name="ht", bufs=2) as htpool,
            tc.tile_pool(name="psep", bufs=1, space="PSUM") as peppool,
        ):
            ilo = cpool.tile([128, CHUNKS * LO_N // 16], I16)
            ihi = cpool.tile([128, CHUNKS * HI_N // 16], I16)
            iota = cpool.tile([128, 128], BF16)
            drel = cpool.tile([128, NTILES], F32)
            ae = cpool.tile([128, NTILES, H1], BF16)
            b1 = cpool.tile([128, FMID], BF16)
            ident = cpool.tile([128, 128], BF16)
            w2c = cpool.tile([FMID, FOUT + 2], BF16)
            # per-chunk JIT input loads: chunk 0 up front, chunk ch+1 during
            # chunk ch, so the big arrays never collide with early gathers
            lc, hc = LO_N // 16, HI_N // 16

            def load_chunk_inputs(c):
                nc.sync.dma_start(out=ilo[:, c * lc:(c + 1) * lc],
                                  in_=ilo_in[:, c * lc:(c + 1) * lc])
                nc.sync.dma_start(out=ihi[:, c * hc:(c + 1) * hc],
                                  in_=ihi_in[:, c * hc:(c + 1) * hc])
                nc.sync.dma_start(out=drel[:, c * TPC:(c + 1) * TPC],
                                  in_=drel_in[:, c * TPC:(c + 1) * TPC])
                nc.sync.dma_start(out=ae[:, c * TPC:(c + 1) * TPC, :],
                                  in_=ae_in[:, c * TPC:(c + 1) * TPC, :])

            load_chunk_inputs(0)
            nc.sync.dma_start(out=iota[:], in_=iota_in[:, :])
            load_chunk_inputs(1)
            nc.sync.dma_start(out=ilo[:, 2 * lc:], in_=ilo_in[:, 2 * lc:])
            nc.sync.dma_start(out=ihi[:, 2 * hc:], in_=ihi_in[:, 2 * hc:])
            nc.sync.dma_start(out=drel[:, 2 * TPC:], in_=drel_in[:, 2 * TPC:])
            nc.sync.dma_start(out=ae[:, 2 * TPC:, :], in_=ae_in[:, 2 * TPC:, :])
            nc.sync.dma_start(out=b1[:], in_=b1_in[:, :])
            nc.sync.dma_start(out=ident[:], in_=id_in[:, :])
            nc.sync.dma_start(out=w2c[:], in_=w2c_in[:, :])

            def epilogue(ch, accP, w0=0, w1=CHUNK_W):
                # batched normalize + bias + ELU over windows [w0, w1)
                nw = w1 - w0
                sEps = epool.tile([128, nw, H1], F32, name=f"sEps{nw}")
                nc.scalar.activation(out=sEps[:, :, :],
                                     in_=accP[:, w0:w1, 128:128 + H1],
                                     func=ACTF.Copy, bias=1e-30)
                rec = epool.tile([128, nw, H1], BF16, name=f"rec{nw}")
                with nc.allow_low_precision(reason="coef normalize in bf16"):
                    nc.vector.reciprocal(out=rec[:, :, :], in_=sEps[:, :, :])
                hB = epool.tile([128, nw, 128], BF16, name=f"hB{nw}")
                nc.vector.tensor_tensor(
                    out=hB[:, :, :].rearrange("p w (c h) -> p w c h", h=H1),
                    in0=accP[:, w0:w1, 0:128].rearrange(
                        "p w (c h) -> p w c h", h=H1),
                    in1=rec[:, :, :].unsqueeze(2).broadcast_to(
                        [128, nw, C1, H1]),
                    op=ALU.mult)
                nc.vector.tensor_tensor(
                    out=hB[:, :, :], in0=hB[:, :, :],
                    in1=b1[:].unsqueeze(1).broadcast_to([128, nw, 128]),
                    op=ALU.add)
                # exp(min(x,0)) = exp(-relu(-x)): both steps on ACT
                tmp = epool.tile([128, nw, 128], BF16, name=f"tmp{nw}")
                nc.scalar.activation(out=tmp[:, :, :], in_=hB[:, :, :],
                                     func=ACTF.Relu, scale=-1.0)
                nc.scalar.activation(out=tmp[:, :, :], in_=tmp[:, :, :],
                                     func=ACTF.Exp, scale=-1.0)
                helu = hpool.tile([128, nw, 128], BF16, name=f"helu{nw}")
                nc.vector.tensor_scalar(out=helu[:, :, :], in0=tmp[:, :, :],
                                        scalar1=-1.0, scalar2=None, op0=ALU.add)
                nc.vector.tensor_tensor(out=helu[:, :, :], in0=helu[:, :, :],
                                        in1=hB[:, :, :], op=ALU.max)
                # layer-2 features: [h @ W2 | h @ W2A] via batched PE transpose
                accA = accapool.tile([128, nw, FOUT + 2], BF16,
                                     name=f"accA{nw}")
                psT = peppool.tile([128, nw, 128], BF16, space="PSUM",
                                   name=f"psT{nw}")
                for wi in range(nw):
                    nc.tensor.transpose(out=psT[:, wi, :], in_=helu[:, wi, :],
                                        identity=ident[:])
                hT = htpool.tile([128, nw, 128], BF16, name=f"hT{nw}")
                nc.scalar.copy(out=hT[:, :, :], in_=psT[:, :, :])
                n1 = (nw + 1) // 2
                psA1 = peppool.tile([128, n1, FOUT + 2], F32, space="PSUM",
                                    name=f"psA1{nw}")
                psA2 = peppool.tile([128, max(nw - n1, 1), FOUT + 2], F32,
                                    space="PSUM", name=f"psA2{nw}")
                for wi in range(nw):
                    pa = psA1[:, wi, :] if wi < n1 else psA2[:, wi - n1, :]
                    nc.tensor.matmul(out=pa, lhsT=hT[:, wi, :], rhs=w2c[:],
                                     start=True, stop=True)
                nc.scalar.copy(out=accA[:, 0:n1, :], in_=psA1[:, :, :])
                if nw > n1:
                    nc.scalar.copy(out=accA[:, n1:nw, :], in_=psA2[:, :, :])
                c0 = (ch * CHUNK_W + w0) * (FOUT + 2)
                c1 = (ch * CHUNK_W + w1) * (FOUT + 2)
                nc.sync.dma_start(out=dump_out[:, c0:c1], in_=accA[:, :, :])

            nlo_t = CHUNK_W * LOT

            def emit_exp(c):
                # e = exp(leaky_relu(alpha)) on ACT (alpha host-preadded);
                # emitted one chunk ahead so the in-order ACT queue never
                # stalls it behind the current chunk's psum copies
                A2 = apool.tile([128, TPC, H1], BF16)
                RHS = rpool.tile([128, TPC, 128 + H1], BF16)
                nc.scalar.activation(out=A2[:, :, :],
                                     in_=ae[:, c * TPC:(c + 1) * TPC, :],
                                     func=ACTF.Prelu, alpha=NEG_SLOPE)
                nc.scalar.activation(out=RHS[:, :, 128:128 + H1],
                                     in_=A2[:, :, :], func=ACTF.Exp)
                return RHS

            prev = None
            RHS_cur = None
            for ch in range(CHUNKS):
                t0 = ch * TPC
                G = gpool.tile([128, TPC, 128], BF16)
                _emit_gathers(nc, G, table_in[:, :], ilo, 0, nlo_t,
                              ch * (LO_N // 16))
                _emit_gathers(nc, G, table_in[HI_BASE:, :], ihi, nlo_t,
                              TPC - nlo_t, ch * (HI_N // 16))
                if ch == 0:
                    RHS_cur = emit_exp(0)
                RHS_nxt = emit_exp(ch + 1) if ch + 1 < CHUNKS else None
                RHS = RHS_cur
                # previous chunk's epilogue: its deps are long done, so the
                # in-order DVE queue never stalls on it, and it fills DVE
                # while this chunk's gathers land
                if prev is not None:
                    epilogue(*prev)
                def emit_msg(lo0, n):
                    # msg = xh[src] * e (broadcast over channels; c-major)
                    in0 = G[:, lo0:lo0 + n, :].rearrange(
                        "p t (c h) -> p t c h", h=H1)
                    in1 = RHS[:, lo0:lo0 + n, 128:128 + H1].unsqueeze(
                        2).broadcast_to([128, n, C1, H1])
                    out0 = RHS[:, lo0:lo0 + n, 0:128].rearrange(
                        "p t (c h) -> p t c h", h=H1)
                    nc.vector.tensor_tensor(out=out0, in0=in0, in1=in1,
                                            op=ALU.mult)

                emit_msg(0, 16)             # lo tiles, gather-piece aligned
                emit_msg(16, nlo_t - 16)
                accP = accppool.tile([128, CHUNK_W, 128 + H1], BF16)
                for wi in range(CHUNK_W):
                    Ss = []
                    for t in range(TPW):
                        if t < LOT:
                            g = wi * LOT + t
                        else:
                            g = CHUNK_W * LOT + wi * HIT + (t - LOT)
                        S = spool.tile([128, 128], BF16)
                        nc.vector.tensor_scalar(
                            out=S[:], in0=iota[:],
                            scalar1=drel[:, t0 + g:t0 + g + 1], scalar2=None,
                            op0=ALU.is_equal)
                        Ss.append((g, S))
                    if wi == 0:
                        emit_msg(nlo_t, 16)            # hi tiles, two pieces
                        emit_msg(nlo_t + 16, TPC - nlo_t - 16)
                    psum = ppool.tile([128, 128 + H1], F32, space="PSUM")
                    for t, (g, S) in enumerate(Ss):
                        nc.tensor.matmul(out=psum[:], lhsT=S[:],
                                         rhs=RHS[:, g, :],
                                         start=(t == 0), stop=(t == TPW - 1))
                    nc.scalar.copy(out=accP[:, wi, :], in_=psum[:])
                prev = (ch, accP)
                RHS_cur = RHS_nxt
            # final epilogue in two overlapping halves: short chains pipeline
            # instead of one deep serial chain in the drain (window 3 is
            # computed twice with identical results)
            epilogue(prev[0], prev[1], 0, 3)
            epilogue(prev[0], prev[1], 2, 5)
    nc.compile()
    return nc


def build_E2():
    lay = LAY2
    CHUNKS, TPC, LO_N, HI_N, NTILES = (lay.CHUNKS, lay.TPC, lay.LO_N,
                                       lay.HI_N, lay.NTILES)
    CHUNK_W, WINS, WIN_NODES = lay.CW, lay.WINS, lay.WIN
    nc = _new_nc()
    table_in = nc.declare_dram_parameter("table", [N, 256], BF16, isOutput=False)
    ae_in = nc.declare_dram_parameter("ae", [128, NTILES, 1], BF16,
                                      isOutput=False)
    ilo_in = nc.declare_dram_parameter("ilo", [128, CHUNKS * LO_N // 16], I16,
                                       isOutput=False)
    ihi_in = nc.declare_dram_parameter("ihi", [128, CHUNKS * HI_N // 16], I16,
                                       isOutput=False)
    drel_in = nc.declare_dram_parameter("drel", [128, NTILES], F32, isOutput=False)
    iota_in = nc.declare_dram_parameter("iota", [128, 128], BF16, isOutput=False)
    b2_in = nc.declare_dram_parameter("b2rep", [128, FOUT], BF16, isOutput=False)
    out_out = nc.declare_dram_parameter("out", [NPC, FOUT], BF16, isOutput=True)

    with tile.TileContext(nc) as tc:
        with (
            tc.tile_pool(name="const", bufs=1) as cpool,
            tc.tile_pool(name="gat", bufs=2) as gpool,
            tc.tile_pool(name="alp", bufs=2) as apool,
            tc.tile_pool(name="sel", bufs=24) as spool,
            tc.tile_pool(name="psw", bufs=7, space="PSUM") as ppool,
            tc.tile_pool(name="agg", bufs=2) as aggpool,
            tc.tile_pool(name="rc", bufs=4) as rcpool,
        ):
            ilo = cpool.tile([128, CHUNKS * LO_N // 16], I16)
            ihi = cpool.tile([128, CHUNKS * HI_N // 16], I16)
            iota = cpool.tile([128, 128], BF16)
            drel = cpool.tile([128, NTILES], F32)
            ae = cpool.tile([128, NTILES, 1], BF16)
            b2 = cpool.tile([128, FOUT], BF16)
            lc, hc = LO_N // 16, HI_N // 16

            def load_chunk_inputs(c):
                nc.sync.dma_start(out=ilo[:, c * lc:(c + 1) * lc],
                                  in_=ilo_in[:, c * lc:(c + 1) * lc])
                nc.sync.dma_start(out=ihi[:, c * hc:(c + 1) * hc],
                                  in_=ihi_in[:, c * hc:(c + 1) * hc])
                nc.sync.dma_start(out=drel[:, c * TPC:(c + 1) * TPC],
                                  in_=drel_in[:, c * TPC:(c + 1) * TPC])
                nc.sync.dma_start(out=ae[:, c * TPC:(c + 1) * TPC, :],
                                  in_=ae_in[:, c * TPC:(c + 1) * TPC, :])

            load_chunk_inputs(0)
            nc.sync.dma_start(out=iota[:], in_=iota_in[:, :])
            load_chunk_inputs(1)
            nc.sync.dma_start(out=ilo[:, 2 * lc:], in_=ilo_in[:, 2 * lc:])
            nc.sync.dma_start(out=ihi[:, 2 * hc:], in_=ihi_in[:, 2 * hc:])
            nc.sync.dma_start(out=drel[:, 2 * TPC:], in_=drel_in[:, 2 * TPC:])
            nc.sync.dma_start(out=ae[:, 2 * TPC:, :], in_=ae_in[:, 2 * TPC:, :])
            nc.sync.dma_start(out=b2[:], in_=b2_in[:, :])

            for ch in range(CHUNKS):
                t0 = ch * TPC
                nlo_t = CHUNK_W * LOT
                G = gpool.tile([128, TPC, 256], BF16)
                _emit_gathers_il(nc, G, table_in[:, :], table_in[HI_BASE:, :],
                                 ilo, ihi, nlo_t, TPC - nlo_t,
                                 ch * (LO_N // 16), ch * (HI_N // 16))
                A = apool.tile([128, TPC, 1], BF16)
                A2 = apool.tile([128, TPC, 1], F32)
                nc.scalar.activation(out=A[:, :, :],
                                     in_=ae[:, t0:t0 + TPC, :],
                                     func=ACTF.Prelu, alpha=NEG_SLOPE)
                nc.scalar.activation(out=A2[:, :, :], in_=A[:, :, :],
                                     func=ACTF.Exp)
                aggN = aggpool.tile([128, CHUNK_W, FOUT], BF16)
                for wi in range(CHUNK_W):
                    Ss = []
                    for t in range(TPW):
                        if t < LOT:
                            g = wi * LOT + t
                        else:
                            g = CHUNK_W * LOT + wi * HIT + (t - LOT)
                        S = spool.tile([128, 128], BF16)
                        nc.vector.tensor_scalar(
                            out=S[:], in0=iota[:],
                            scalar1=drel[:, t0 + g:t0 + g + 1],
                            scalar2=A2[:, g, 0:1],
                            op0=ALU.is_equal, op1=ALU.mult)
                        Ss.append((g, S))
                    psum = ppool.tile([128, 129], F32, space="PSUM")
                    for t, (g, S) in enumerate(Ss):
                        nc.tensor.matmul(out=psum[:], lhsT=S[:],
                                         rhs=G[:, g, 0:129],
                                         start=(t == 0), stop=(t == TPW - 1))
                    # out = agg / s: fold 1/s into the ACT psum copy as a
                    # per-partition scale
                    sEps = rcpool.tile([128, 1], F32)
                    nc.scalar.activation(out=sEps[:], in_=psum[:, 128:129],
                                         func=ACTF.Copy, bias=1e-30)
                    rec = rcpool.tile([128, 1], F32)
                    nc.vector.reciprocal(out=rec[:], in_=sEps[:])
                    nc.scalar.activation(out=aggN[:, wi, :], in_=psum[:, 0:128],
                                         func=ACTF.Copy, scale=rec[:])
                accO = aggpool.tile([128, CHUNK_W, FOUT], BF16)
                r0 = ch * CHUNK_W * WIN_NODES
                if ch < CHUNKS - 1:
                    nc.vector.tensor_tensor(
                        out=accO[:, :, :], in0=aggN[:, :, :],
                        in1=b2[:].unsqueeze(1).broadcast_to(
                            [128, CHUNK_W, 128]),
                        op=ALU.add)
                    out_ap = out_out[r0:r0 + CHUNK_W * WIN_NODES, :].rearrange(
                        "(w p) c -> p w c", p=WIN_NODES)
                    nc.sync.dma_start(out=out_ap, in_=accO[:, :, :])
                else:
                    # last chunk: per-window finalize + store so the drain
                    # pipelines; the final window is ragged (106 nodes)
                    rem = NPC - (WINS - 1) * WIN_NODES
                    for wi in range(CHUNK_W):
                        nc.vector.tensor_tensor(
                            out=accO[:, wi, :], in0=aggN[:, wi, :],
                            in1=b2[:], op=ALU.add)
                        w0 = r0 + wi * WIN_NODES
                        nrows = WIN_NODES if wi < CHUNK_W - 1 else rem
                        nc.sync.dma_start(out=out_out[w0:w0 + nrows, :],
                                          in_=accO[0:nrows, wi, :])
    nc.compile()
    return nc


# ----------------------------------------------------------------------------
# Host orchestration
# ----------------------------------------------------------------------------

def _run(nc, in_maps, tag):
    trace = os.environ.get("KERNEL_TRACE", "0") == "1"
    res = run_bass_kernel_spmd(nc, in_maps, list(range(NCORES)), trace=trace)
    if trace:
        _CACHE.setdefault("profiles", {})[tag] = res
    return res.results


def _expand_ae(cores, a_src, a_dst):
    """Host-expanded per-slot alpha = a_src[src] + a_dst[dst] per core."""
    a_src = a_src.astype(np.float32)
    a_dst = a_dst.astype(np.float32)
    return [np.ascontiguousarray(
        (a_src[cd["slot_src"]] + a_dst[cd["slot_dst"]]).astype(BF))
        for cd in cores]


def kernel(x, src, dst, W1, att_src1, att_dst1, b1, W2, att_src2, att_dst2, b2):
    x = np.asarray(x, np.float32)
    src = np.asarray(src, np.int64)
    dst = np.asarray(dst, np.int64)
    W1 = np.asarray(W1, np.float32)
    W2 = np.asarray(W2, np.float32)
    att_src1 = np.asarray(att_src1, np.float32)
    att_dst1 = np.asarray(att_dst1, np.float32)
    att_src2 = np.asarray(att_src2, np.float32)
    att_dst2 = np.asarray(att_dst2, np.float32)
    b1 = np.asarray(b1, np.float32)
    b2 = np.asarray(b2, np.float32)

    key = "progs"
    if key not in _CACHE:
        _CACHE[key] = (build_T(), build_E1(), build_E2())
    ncT, ncE1, ncE2 = _CACHE[key]

    ekey = ("edges", hash(src.tobytes()), hash(dst.tobytes()))
    if ekey not in _CACHE:
        _CACHE[ekey] = (_prep_edges(src, dst, LAY1),
                        _prep_edges(src, dst, LAY2))
    cores1, cores2 = _CACHE[ekey]

    perm = _perm_cmajor()
    W1P = np.ascontiguousarray(W1[:, perm])
    W1A_src = np.einsum("fhc,hc->fh", W1.reshape(FIN, H1, C1), att_src1)
    W1A_dst = np.einsum("fhc,hc->fh", W1.reshape(FIN, H1, C1), att_dst1)
    WT = np.concatenate([W1P, W1A_src, W1A_dst], axis=1).astype(BF)  # [128,144]
    b1P = b1[perm].astype(np.float32)
    W2P = np.ascontiguousarray(W2[perm, :])
    att2cat = np.stack([att_src2[0], att_dst2[0]], axis=1).astype(np.float32)
    W2A = (W2P @ att2cat).astype(np.float32)  # [128, 2] in permuted row space
    W2C = np.concatenate([W2P, W2A], axis=1).astype(BF)  # [128, 130]

    ident = np.eye(128, dtype=np.float32).astype(BF)
    iota = np.tile(np.arange(128, dtype=np.float32), (128, 1)).astype(BF)
    b1rep = np.tile(b1P, (128, 1)).astype(BF)
    b2rep = np.tile(b2, (128, 1)).astype(BF)

    # ---- Launch T: per-core [xh | a_src | a_dst] -------------------------
    xtpad = np.zeros((NCORES, 128, NPC_PAD), BF)
    for c in range(NCORES):
        xtpad[c, :, :NPC] = x[c * NPC:(c + 1) * NPC].T.astype(BF)
    in_maps = [{"xt": xtpad[c], "w": WT} for c in range(NCORES)]
    resT = _run(ncT, in_maps, "T")
    parts = []
    for c in range(NCORES):
        d = resT[c]["dump"].reshape(128, NT_T, TCOLS)
        parts.append(d.transpose(1, 0, 2).reshape(NPC_PAD, TCOLS)[:NPC])
    ta = np.concatenate(parts)                      # [N, 144] bf16
    table1 = np.ascontiguousarray(ta[:, 0:FMID])    # [N, 128] bf16
    a1_src = np.ascontiguousarray(ta[:, FMID:FMID + H1])
    a1_dst = np.ascontiguousarray(ta[:, FMID + H1:FMID + 2 * H1])
    ae1 = _expand_ae(cores1, a1_src, a1_dst)

    # ---- Launch E1 --------------------------------------------------------
    in_maps = [{"table": table1, "ae": ae1[c], "ilo": cores1[c]["idx_lo"],
                "ihi": cores1[c]["idx_hi"], "drel": cores1[c]["drel"],
                "iota": iota, "b1rep": b1rep, "ident": ident, "w2c": W2C}
               for c in range(NCORES)]
    resE1 = _run(ncE1, in_maps, "E1")
    parts = []
    for c in range(NCORES):
        d = resE1[c]["dump"].reshape(128, LAY1.WINS, FOUT + 2)
        parts.append(d.transpose(1, 0, 2)[:, :LAY1.WIN, :].reshape(
            LAY1.WINS * LAY1.WIN, FOUT + 2))
    ha = np.concatenate(parts)                      # [N, 130] bf16
    table2 = np.zeros((N, 256), BF)                 # [xh2 | 1.0 | pad]
    table2[:, 0:FOUT] = ha[:, 0:FOUT]
    table2[:, FOUT] = BF(1.0)
    a2_src = np.ascontiguousarray(ha[:, FOUT:FOUT + 1])
    a2_dst = np.ascontiguousarray(ha[:, FOUT + 1:FOUT + 2])
    ae2 = _expand_ae(cores2, a2_src, a2_dst)

    # ---- Launch E2 --------------------------------------------------------
    in_maps = [{"table": table2, "ae": ae2[c], "ilo": cores2[c]["idx_lo"],
                "ihi": cores2[c]["idx_hi"], "drel": cores2[c]["drel"],
                "iota": iota, "b2rep": b2rep}
               for c in range(NCORES)]
    resE2 = _run(ncE2, in_maps, "E2")
    out = np.concatenate([resE2[c]["out"].astype(np.float32)
                          for c in range(NCORES)])
    return np.ascontiguousarray(out)
